# revision 12
# baseline (speedup 1.0000x reference)
"""Trainium2 Bass kernel for nn_Attn_14078902796904.

Computes attn = softmax(encoder_outputs @ hidden) for
encoder_outputs [65536, 1024] f32, hidden [1024] f32 -> [1, 1, 65536] f32.

Strategy (sequence-parallel across 8 NeuronCores):
  - Core c gets rows [c*8192, (c+1)*8192) of encoder_outputs; hidden is
    replicated (host pre-broadcasts to [128, 1024]).
  - Row -> partition mapping is contiguous: partition p holds rows
    [p*64, (p+1)*64) of the core's shard, so a chunk of nb blocks is one
    contiguous nb*4 KiB descriptor per partition.  32 KiB descriptors
    (8-block / 4 MiB chunks) sustain ~415 GB/s HBM->SBUF vs ~340 GB/s
    for 16 KiB ones; the chunk list tapers at the end so the last
    chunk's compute tail is short.
  - One fused DVE scalar_tensor_tensor per 1024-block computes
    (x * 1) * hidden with accum_out = the row dot; the product is
    discarded into a PSUM scratch tile so the only SBUF traffic is the
    DMA stream write + DVE read.  (SCALAR_TENSOR_TENSOR_ARITH is a
    stock DVE opcode; TENSOR_TENSOR_REDUCE and all GpSimd compute ops
    hang this runtime's ucode.)  energies [128, 64] is padded to 128
    cols so the output DMA writes 512 B per partition, above the DMA
    line-rate minimum.
  - Device returns raw energies; host does the softmax in f64 (it
    already had to recombine per-core partials anyway).
"""

import os
import sys
import time

for _p in ("/opt/trn_rl_repo", "/root/.axon_site/_ro/trn_rl_repo"):
    if os.path.isdir(_p) and _p not in sys.path:
        sys.path.append(_p)

import numpy as np

import concourse.tile as tile
from concourse import bacc, mybir
from concourse.bass_utils import run_bass_kernel_spmd

S = 65536
H = 1024
N_CORES = 8
SC = S // N_CORES          # 8192 rows per core
P = 128                    # partitions
NT = SC // P               # 64 blocks of 128 rows per core
GMAX = 8                   # max blocks per DMA chunk (4 MB)
OUTW = 128                 # out columns (64 real + pad to 512 B/partition)

# chunk sizes in blocks; tapered at BOTH ends: small head chunks so the
# first fused op starts as soon as ~0.5 MB has landed (instead of
# waiting out a full 4 MB transfer), small tail chunks so the last
# op's data dependency is short.
CHUNKS = [1, 1, 1, 1, 2, 2, 4] + [8] * 6 + [2, 1, 1]
assert sum(CHUNKS) == NT

INP_BUFS = 5

_DT = mybir.dt.float32


def _build_nc():
    nc = bacc.Bacc("TRN2", target_bir_lowering=False, debug=False,
                   enable_asserts=False, num_devices=N_CORES)
    enc = nc.dram_tensor("enc", [SC, H], _DT, kind="ExternalInput")
    hid = nc.dram_tensor("hid", [P, H], _DT, kind="ExternalInput")
    out = nc.dram_tensor("out", [P, OUTW], _DT, kind="ExternalOutput")

    # enc_r[p, n, h] = enc[p*NT + n, h]: per-partition contiguous rows
    enc_r = enc.ap().rearrange("(p n) h -> p n h", p=P)

    with tile.TileContext(nc) as tc:
        with (
            tc.tile_pool(name="inp", bufs=INP_BUFS) as inp_pool,
            tc.tile_pool(name="consts", bufs=1) as consts,
            tc.tile_pool(name="small", bufs=1) as small,
            tc.psum_pool(name="ps", bufs=1) as ps,
        ):
            # hid staged through SBUF into PSUM so the steady-state DVE
            # traffic on SBUF is only the t_in read (DMA writes SBUF at
            # ~415 GB/s; every SBUF access DVE avoids reduces contention).
            hidstage = consts.tile([P, H], _DT)
            nc.sync.dma_start(hidstage[:], hid.ap())
            hidps = ps.tile([P, H], _DT)
            nc.vector.tensor_copy(hidps[:], hidstage[:])

            scratch = ps.tile([P, H], _DT)
            eps = ps.tile([P, NT], _DT)
            energies = small.tile([P, OUTW], _DT)
            nc.vector.memset(energies[:], 0.0)

            blk = 0
            for nb in CHUNKS:
                t_in = inp_pool.tile([P, GMAX * H], _DT, tag="t_in")
                nc.sync.dma_start(
                    t_in[:, :nb * H].rearrange("p (b h) -> p b h", h=H),
                    enc_r[:, blk:blk + nb, :],
                )
                for j in range(nb):
                    nc.vector.scalar_tensor_tensor(
                        scratch[:],
                        t_in[:, j * H:(j + 1) * H],
                        1.0,
                        hidps[:],
                        op0=mybir.AluOpType.mult,
                        op1=mybir.AluOpType.mult,
                        accum_out=eps[:, blk + j:blk + j + 1],
                    )
                blk += nb

            nc.vector.tensor_copy(energies[:, :NT], eps[:])
            nc.sync.dma_start(out.ap(), energies[:])
    nc.compile()
    return nc


_NC_CACHE = None


def _get_nc():
    global _NC_CACHE
    if _NC_CACHE is None:
        _NC_CACHE = _build_nc()
    return _NC_CACHE


def run_device(hidden, encoder_outputs, **spmd_kwargs):
    """Run the per-core kernels; returns (list of per-core result dicts,
    BassKernelResults)."""
    hidden = np.asarray(hidden, dtype=np.float32)
    encoder_outputs = np.asarray(encoder_outputs, dtype=np.float32)
    hidrep = np.ascontiguousarray(np.broadcast_to(hidden, (P, H)))
    in_maps = [
        {
            "enc": np.ascontiguousarray(encoder_outputs[c * SC:(c + 1) * SC]),
            "hid": hidrep,
        }
        for c in range(N_CORES)
    ]
    # The axon-proxied runtime occasionally reports the accelerator as
    # unrecoverable and then recovers on the next attempt; retry.
    last_err = None
    for attempt in range(3):
        try:
            res = run_bass_kernel_spmd(
                _get_nc(), in_maps, list(range(N_CORES)), **spmd_kwargs
            )
            return res.results, res
        except Exception as e:  # noqa: BLE001
            last_err = e
            time.sleep(2.0)
    raise last_err


def combine(results):
    """Host softmax over the concatenated per-core energies -> [1,1,S]."""
    # energies[core, p, t] = dot(enc_row core*SC + p*NT + t), so a C-order
    # flatten of the first NT columns is already sequence order.
    e = np.stack([r["out"][:, :NT] for r in results])
    e = e.astype(np.float64).reshape(S)
    e -= e.max()
    w = np.exp(e)
    attn = w / w.sum()
    return attn.astype(np.float32)[None, None, :]


def kernel(hidden, encoder_outputs):
    results, _ = run_device(hidden, encoder_outputs)
    return combine(results)
